# revision 14
# baseline (speedup 1.0000x reference)
"""BitLinear (RMSNorm + ternary-quantized linear) on 8 trn2 NeuronCores.

Reference math (fp32):
    xn   = x * rsqrt(mean(x^2, -1) + 1e-5) * gamma          # [B,S,K]
    s    = max(mean(|w|), 1e-5)                             # scalar
    q    = round(clip(w / s, -1, 1))                        # ternary {-1,0,1}
    out  = (xn @ q.T) * s                                   # [B,S,Dout]

Identities used by the kernel:
    q = (w > s/2) - (w < -s/2)   (exact, incl. round-half-even at |wn|=0.5)
    out[t,o] = inv[t] * s * sum_k (x[t,k]*gamma[k]) * q[o,k]
so gamma folds into x before the fp16 cast, q is exactly ternary in fp16,
and inv*s folds into the PSUM->SBUF epilogue. The contraction runs on the
PE in fp16 with fp32 PSUM accumulation.

Sharding: 2 token-groups x 4 dout-groups (core = rg*4 + cg).

Scale path (the startup critical path): each core's weight shard w_shT
[K, DOUT_SH] is split into an "a" half (k-tiles 0..7) and a "b" half
(k-tiles 8..15) such that the a-halves across the 8 cores partition the
FULL weight exactly once: rg=0 cores own original k [0,1024), rg=1 cores
own k [1024,2048). For rg=1 cores the host swaps the k-halves of x_sh,
gamma and w_shT so the program is identical on every core (the
contraction is invariant under a shared permutation of k). The a-half is
DMA'd first, |w| partial-reduced on DVE while it streams, AllReduced
(triggered from an otherwise-empty gpsimd queue at ~27us), and kept
resident in SBUF so quantization of k-tiles 0..7 needs no second read.

Main loop: per strip j (128 tokens), loop k-tile t outer / dout-chunk d
inner so one stationary xT load serves 4 N=512 matmuls (LDWEIGHTS
amortization). 8 PSUM banks = 2 strips in flight; drain via ACT copy
scaled by inv[j]*s; output DMA'd from gpsimd.

Optional fp8 split (NK8 > 0): k-tiles 0..NK8-1 run as e4m3 DoubleRow
pair-matmuls (2 k-tiles per stationary load, 2 MACs/cell/cycle), the
rest stay fp16. x and q for those tiles are exact-in-fp8 ternary /
rounded activations; the added output error was measured offline.
"""

import numpy as np
import ml_dtypes

import concourse.bass as bass
import concourse.tile as tile
from concourse import bacc, mybir
from concourse.bass_utils import run_bass_kernel_spmd

F32 = mybir.dt.float32
F16 = mybir.dt.float16
BF16 = mybir.dt.bfloat16
F8 = mybir.dt.float8e4

# Full-problem constants
B, S, K, DOUT = 4, 2048, 2048, 8192
N_CORES = 8
RG, CG = 2, 4  # token groups x dout groups
TOK_SH = (B * S) // RG     # 4096 tokens per core
DOUT_SH = DOUT // CG       # 2048 out-features per core
KT = K // 128              # 16 contraction tiles
KA = KT // 2               # tiles held + |w|-reduced per core (the "a" half)
N_STRIP = TOK_SH // 128    # 32 token strips
EPS = 1e-5
W_COUNT = float(DOUT * K)  # 16777216 = 2**24 (exact power of two)

NK8 = 8  # leading k-tiles computed in fp8 DoubleRow (0 = pure fp16)


def build_nc(nk8=NK8, use_cc=True, debug_fixed_scale=None):
    """Build the SPMD Bass program (one program, per-core inputs differ)."""
    nc = bacc.Bacc("TRN2", target_bir_lowering=False, num_devices=N_CORES)

    x_d = nc.declare_dram_parameter("x_sh", [TOK_SH, K], BF16, isOutput=False)
    wa_d = nc.declare_dram_parameter("w_a", [KA * 128, DOUT_SH], F32, isOutput=False)
    wb_d = nc.declare_dram_parameter("w_b", [(KT - KA) * 128, DOUT_SH], F32, isOutput=False)
    out_d = nc.declare_dram_parameter("out_sh", [TOK_SH, DOUT_SH], F32, isOutput=True)

    with tile.TileContext(nc, num_cores=N_CORES) as tc:
        with (
            tc.tile_pool(name="consts", bufs=1) as consts,
            tc.tile_pool(name="wheld", bufs=1) as wheld,
            tc.tile_pool(name="wstream", bufs=1) as wstream,
            tc.tile_pool(name="f32s", bufs=1) as f32s,
            tc.tile_pool(name="f16s", bufs=1) as f16s,
            tc.tile_pool(name="qt", bufs=1) as qtp,
            tc.tile_pool(name="outp", bufs=1) as outp,
            tc.tile_pool(name="psum", bufs=8, space="PSUM") as psum,
            tc.tile_pool(name="dram", bufs=1, space="DRAM") as dram,
        ):
            # ---- constants ------------------------------------------------
            # cblock cols: 0 ones, 1 eps, 2 prev, 3 allv, 4.. parts
            cblock = consts.tile([128, 4 + KA], F32)
            ones_col = cblock[:, 0:1]
            eps_t = cblock[:, 1:2]
            prev = cblock[:, 2:3]
            allv = cblock[:, 3:4]
            parts = cblock[:, 4:4 + KA]
            nc.vector.memset(ones_col, 1.0)
            nc.vector.memset(eps_t, EPS)
            ones_row = consts.tile([1, 128], F32)
            nc.vector.memset(ones_row, 1.0)
            # sblock cols: 0 s_mean, 1 s_clip, 2 s_bc, 3 t_bc, 4 nt_bc
            sblock = consts.tile([128, 5], F32)
            invb = consts.tile([128, N_STRIP], F32)  # inv[j]*s per strip

            s_bc = sblock[:, 2:3]

            # ---- phase W1: |w| partial over the held a-half ---------------
            # wa loads go FIRST on sync so their transfers win the DMA queues;
            # the partial reduce keeps pace on DVE; gpsimd stays empty so the
            # collective triggers the moment the partial lands.
            wa = wheld.tile([128, KA, DOUT_SH], F32)
            for i in range(KA):
                eng = nc.sync if i % 2 == 0 else nc.scalar
                eng.dma_start(out=wa[:, i, :],
                              in_=wa_d[i * 128:(i + 1) * 128, :])
            for i in range(KA):
                nc.vector.tensor_reduce(
                    parts[:, i:i + 1], wa[:, i, :],
                    axis=mybir.AxisListType.X,
                    op=mybir.AluOpType.add, apply_absolute_value=True)
            nc.vector.tensor_reduce(prev, parts, axis=mybir.AxisListType.X,
                                    op=mybir.AluOpType.add)


            # ---- strip prep ----------------------------------------------
            # xf DMA dispatch on sync AFTER cc_in so the wa transfers own the
            # DMA queues during W1. x16 does not depend on inv: the rmsnorm
            # chain (square/sqrt/recip) only feeds the drain scale invb[j].
            xT_tiles = {}
            deferred_inv = []

            def prep_strip(j, defer_inv=False):
                xb = f32s.tile([128, K], BF16, tag="xb", bufs=3, name=f"xb{j}")
                nc.sync.dma_start(out=xb, in_=x_d[j * 128:(j + 1) * 128, :])
                xsq = f32s.tile([128, K], mybir.dt.float8e5, tag="junk",
                                bufs=1, name=f"xsq{j}")
                sc = f32s.tile([128, 3], F32, tag="sc", bufs=8, name=f"sc{j}")
                ssq, rms, inv = sc[:, 0:1], sc[:, 1:2], sc[:, 2:3]
                nc.scalar.activation(xsq, xb,
                                     mybir.ActivationFunctionType.Square,
                                     accum_out=ssq)
                nc.scalar.activation(rms, ssq,
                                     mybir.ActivationFunctionType.Sqrt,
                                     bias=eps_t, scale=1.0 / K)
                nc.vector.reciprocal(inv, rms)
                if defer_inv:
                    # s_bc is not written yet at this emission point; the
                    # multiply must be emitted after the scale chain or it
                    # reads uninitialized SBUF on the first execution.
                    deferred_inv.append((inv, j))
                else:
                    nc.vector.tensor_tensor(invb[:, j:j + 1], inv, s_bc,
                                            mybir.AluOpType.mult)
                xT = f16s.tile([128, KT, 128], BF16, tag="xT", bufs=6,
                               name=f"xT{j}")
                nc.scalar.dma_start_transpose(out=xT, in_=xb)
                xT_tiles[j] = xT

            for j in range(6):
                prep_strip(j, defer_inv=True)

            # collective: cc_in upload on sync (behind only wa + 6 xb loads),
            # trigger + readback on the otherwise-idle gpsimd queue
            cc_in = dram.tile([128, 1], F32)
            cc_out = dram.tile([128, 1], F32, addr_space="Shared")
            nc.sync.dma_start(out=cc_in, in_=prev)
            if use_cc:
                nc.gpsimd.collective_compute(
                    "AllReduce", mybir.AluOpType.add,
                    replica_groups=[list(range(N_CORES))],
                    ins=[cc_in.opt()], outs=[cc_out.opt()],
                )
            else:
                nc.gpsimd.dma_start(out=cc_out, in_=cc_in)

            # ---- scale chain: s = max(sum/2^24, eps), thresholds +-s/2 ----
            # high_priority: the scheduler must slot these at the FRONT of
            # their engines' static order, not behind strip squares.
            prio = tc.high_priority()
            prio.__enter__()
            nc.gpsimd.dma_start(out=allv, in_=cc_out)
            if debug_fixed_scale is not None:
                nc.vector.memset(sblock[:, 2:3], debug_fixed_scale)
                nc.vector.memset(sblock[:, 3:4], debug_fixed_scale * 0.5)
                nc.vector.memset(sblock[:, 4:5], -debug_fixed_scale * 0.5)
            else:
                tot_ps = psum.tile([1, 1], F32, tag="mm")
                nc.tensor.matmul(tot_ps, lhsT=allv, rhs=ones_col,
                                 start=True, stop=True)
                nc.scalar.activation(sblock[0:1, 0:1], tot_ps,
                                     mybir.ActivationFunctionType.Copy,
                                     scale=1.0 / W_COUNT)
                nc.vector.tensor_scalar_max(sblock[0:1, 1:2],
                                            sblock[0:1, 0:1], EPS)
                s_bc_ps = psum.tile([128, 1], F32, tag="mm")
                nc.tensor.matmul(s_bc_ps, lhsT=ones_row, rhs=sblock[0:1, 1:2],
                                 start=True, stop=True)
                nc.scalar.copy(sblock[:, 2:3], s_bc_ps)
                nc.scalar.mul(sblock[:, 3:4], sblock[:, 2:3], 0.5)
                nc.scalar.mul(sblock[:, 4:5], sblock[:, 2:3], -0.5)
            t_bc = sblock[:, 3:4]
            nt_bc = sblock[:, 4:5]
            prio.__exit__(None, None, None)
            for inv, j in deferred_inv:
                nc.vector.tensor_tensor(invb[:, j:j + 1], inv, s_bc,
                                        mybir.AluOpType.mult)
            deferred_inv.clear()

            # ---- quantize: q = (w > s/2) - (w < -s/2) ---------------------
            # pos + subtract on DVE, neg compare on GPSIMD, per-tile.
            # Tiles 0..KA-1 come from SBUF (wa); the rest stream via a
            # rotating pool whose first few DMAs prefetch during the
            # collective window.
            qQ = (qtp.tile([128, KT - nk8, DOUT_SH], BF16, name="qQ")
                  if nk8 < KT else None)
            qQ8 = (qtp.tile([128, nk8, DOUT_SH], F8, name="qQ8")
                   if nk8 > 0 else None)

            def q_dst(t):
                return qQ8[:, t, :] if t < nk8 else qQ[:, t - nk8, :]

            for t in range(KT):
                if t < KA:
                    src = wa[:, t, :]
                else:
                    wt = wstream.tile([128, DOUT_SH], F32, tag="wb", bufs=2,
                                      name=f"wb{t}")
                    nc.sync.dma_start(
                        out=wt, in_=wb_d[(t - KA) * 128:(t - KA + 1) * 128, :])
                    src = wt
                pos = f16s.tile([128, DOUT_SH], BF16, tag="pos", bufs=2,
                                name=f"pos{t}")
                nc.vector.tensor_scalar(pos, src, t_bc, None,
                                        mybir.AluOpType.is_gt)
                neg = f16s.tile([128, DOUT_SH], BF16, tag="neg", bufs=2,
                                name=f"neg{t}")
                nc.vector.tensor_scalar(neg, src, nt_bc, -1.0,
                                        mybir.AluOpType.is_lt,
                                        mybir.AluOpType.mult)
                eng = nc.vector if t % 2 == 0 else nc.gpsimd
                if t < nk8:
                    qb = f16s.tile([128, DOUT_SH], BF16, tag="qb", bufs=2,
                                   name=f"qb{t}")
                    eng.tensor_tensor(qb, pos, neg, mybir.AluOpType.add)
                    nc.vector.tensor_copy(out=q_dst(t), in_=qb)
                else:
                    eng.tensor_tensor(q_dst(t), pos, neg,
                                      mybir.AluOpType.add)

            # ---- main loop: (strip, k-tile, dout-chunk) -------------------
            ND = DOUT_SH // 512  # 4 dout chunks of 512
            for j in range(N_STRIP):
                xT = xT_tiles[j]
                x8T = None
                if nk8 > 0:
                    x8T = f16s.tile([128, nk8 // 2, 2, 128], F8, tag="x8T",
                                    bufs=4, name=f"x8T{j}")
                    nc.vector.tensor_copy(out=x8T[:, :, :, :],
                                          in_=xT[:, 0:nk8, :])
                ps_d = [psum.tile([128, 512], F32, tag="mm",
                                  name=f"ps{j}_{d}") for d in range(ND)]
                # fp8 DoubleRow pair-matmuls for k-tiles [0, nk8):
                # lhsT [128,2,128] (2 k-tiles packed), rhs [128,2,512]
                # (free 1024 @ 2 elem/cycle), out [128,512] = one bank.
                for p2 in range(nk8 // 2):
                    for d in range(ND):
                        nc.tensor.matmul(
                            ps_d[d],
                            lhsT=x8T[:, p2, :, :],
                            rhs=qQ8[:, 2 * p2:2 * p2 + 2,
                                    d * 512:(d + 1) * 512],
                            start=(p2 == 0), stop=False,
                            perf_mode=mybir.MatmulPerfMode.DoubleRow,
                            skip_group_check=True)
                # fp16 for the rest
                for t in range(nk8, KT):
                    for d in range(ND):
                        nc.tensor.matmul(
                            ps_d[d], lhsT=xT[:, t, :],
                            rhs=qQ[:, t - nk8, d * 512:(d + 1) * 512],
                            start=(t == 0 and nk8 == 0), stop=(t == KT - 1),
                            skip_group_check=(nk8 > 0))
                for d in range(ND):
                    ob = outp.tile([128, 512], F32, tag="ob", bufs=4,
                                   name=f"ob{j}_{d}")
                    nc.scalar.activation(
                        out=ob, in_=ps_d[d],
                        func=mybir.ActivationFunctionType.Copy,
                        scale=invb[:, j:j + 1])
                    nc.scalar.dma_start(
                        out=out_d[j * 128:(j + 1) * 128,
                                  d * 512:(d + 1) * 512],
                        in_=ob)
                if j + 6 < N_STRIP:
                    prep_strip(j + 6)

    nc.compile()
    return nc


_NC_CACHE = {}


def _get_nc():
    if "nc" not in _NC_CACHE:
        _NC_CACHE["nc"] = build_nc()
    return _NC_CACHE["nc"]


def make_in_maps(x, weight, gamma):
    """Shard FULL inputs; k-halves swapped for rg=1 so the a-halves of
    w_shT partition the full weight exactly across the 8 cores. gamma is
    folded into x during sharding and the activations staged as bf16 (the
    on-device contraction runs in bf16 anyway)."""
    xf = np.asarray(x, np.float32).reshape(B * S, K)
    gamma = np.asarray(gamma, np.float32)
    xg = (xf * gamma).astype(ml_dtypes.bfloat16)
    wT = np.asarray(weight, np.float32).T  # [K, DOUT]
    half = K // 2
    in_maps = []
    for c in range(N_CORES):
        rg, cg = c // CG, c % CG
        wsh = wT[:, cg * DOUT_SH:(cg + 1) * DOUT_SH]
        xc = xg[rg * TOK_SH:(rg + 1) * TOK_SH]
        if rg == 0:
            wa, wb = wsh[:half], wsh[half:]
        else:
            wa, wb = wsh[half:], wsh[:half]
            xc = np.concatenate([xc[:, half:], xc[:, :half]], axis=1)
        in_maps.append({
            "x_sh": np.ascontiguousarray(xc),
            "w_a": np.ascontiguousarray(wa),
            "w_b": np.ascontiguousarray(wb),
        })
    return in_maps


def kernel(x, weight, gamma):
    in_maps = make_in_maps(x, weight, gamma)
    nc = _get_nc()
    res = run_bass_kernel_spmd(nc, in_maps, list(range(N_CORES))).results

    out = np.empty((B * S, DOUT), dtype=np.float32)
    for c in range(N_CORES):
        rg, cg = c // CG, c % CG
        out[rg * TOK_SH:(rg + 1) * TOK_SH,
            cg * DOUT_SH:(cg + 1) * DOUT_SH] = res[c]["out_sh"]
    return out.reshape(B, S, DOUT)


# revision 15
# speedup vs baseline: 1.0107x; 1.0107x over previous
"""BitLinear (RMSNorm + ternary-quantized linear) on 8 trn2 NeuronCores.

Reference math (fp32):
    xn   = x * rsqrt(mean(x^2, -1) + 1e-5) * gamma          # [B,S,K]
    s    = max(mean(|w|), 1e-5)                             # scalar
    q    = round(clip(w / s, -1, 1))                        # ternary {-1,0,1}
    out  = (xn @ q.T) * s                                   # [B,S,Dout]

Identities used by the kernel:
    q = (w > s/2) - (w < -s/2)   (exact, incl. round-half-even at |wn|=0.5)
    out[t,o] = inv[t] * s * sum_k (x[t,k]*gamma[k]) * q[o,k]
so gamma folds into x before the fp16 cast, q is exactly ternary in fp16,
and inv*s folds into the PSUM->SBUF epilogue. The contraction runs on the
PE in fp16 with fp32 PSUM accumulation.

Sharding: 2 token-groups x 4 dout-groups (core = rg*4 + cg).

Scale path (the startup critical path): each core's weight shard w_shT
[K, DOUT_SH] is split into an "a" half (k-tiles 0..7) and a "b" half
(k-tiles 8..15) such that the a-halves across the 8 cores partition the
FULL weight exactly once: rg=0 cores own original k [0,1024), rg=1 cores
own k [1024,2048). For rg=1 cores the host swaps the k-halves of x_sh,
gamma and w_shT so the program is identical on every core (the
contraction is invariant under a shared permutation of k). The a-half is
DMA'd first, |w| partial-reduced on DVE while it streams, AllReduced
(triggered from an otherwise-empty gpsimd queue at ~27us), and kept
resident in SBUF so quantization of k-tiles 0..7 needs no second read.

Main loop: per strip j (128 tokens), loop k-tile t outer / dout-chunk d
inner so one stationary xT load serves 4 N=512 matmuls (LDWEIGHTS
amortization). 8 PSUM banks = 2 strips in flight; drain via ACT copy
scaled by inv[j]*s; output DMA'd from gpsimd.

Optional fp8 split (NK8 > 0): k-tiles 0..NK8-1 run as e4m3 DoubleRow
pair-matmuls (2 k-tiles per stationary load, 2 MACs/cell/cycle), the
rest stay fp16. x and q for those tiles are exact-in-fp8 ternary /
rounded activations; the added output error was measured offline.
"""

import numpy as np
import ml_dtypes

import concourse.bass as bass
import concourse.tile as tile
from concourse import bacc, mybir
from concourse.bass_utils import run_bass_kernel_spmd

F32 = mybir.dt.float32
F16 = mybir.dt.float16
BF16 = mybir.dt.bfloat16
F8 = mybir.dt.float8e4

# Full-problem constants
B, S, K, DOUT = 4, 2048, 2048, 8192
N_CORES = 8
RG, CG = 2, 4  # token groups x dout groups
TOK_SH = (B * S) // RG     # 4096 tokens per core
DOUT_SH = DOUT // CG       # 2048 out-features per core
KT = K // 128              # 16 contraction tiles
KA = KT // 2               # tiles held + |w|-reduced per core (the "a" half)
N_STRIP = TOK_SH // 128    # 32 token strips
EPS = 1e-5
W_COUNT = float(DOUT * K)  # 16777216 = 2**24 (exact power of two)

NK8 = 8  # leading k-tiles computed in fp8 DoubleRow (0 = pure fp16)


def build_nc(nk8=NK8, use_cc=True, debug_fixed_scale=None):
    """Build the SPMD Bass program (one program, per-core inputs differ)."""
    nc = bacc.Bacc("TRN2", target_bir_lowering=False, num_devices=N_CORES)

    x_d = nc.declare_dram_parameter("x_sh", [TOK_SH, K], BF16, isOutput=False)
    wa_d = nc.declare_dram_parameter("w_a", [KA * 128, DOUT_SH], F32, isOutput=False)
    wb_d = nc.declare_dram_parameter("w_b", [(KT - KA) * 128, DOUT_SH], F32, isOutput=False)
    out_d = nc.declare_dram_parameter("out_sh", [TOK_SH, DOUT_SH], F32, isOutput=True)

    with tile.TileContext(nc, num_cores=N_CORES) as tc:
        with (
            tc.tile_pool(name="consts", bufs=1) as consts,
            tc.tile_pool(name="wheld", bufs=1) as wheld,
            tc.tile_pool(name="wstream", bufs=1) as wstream,
            tc.tile_pool(name="f32s", bufs=1) as f32s,
            tc.tile_pool(name="f16s", bufs=1) as f16s,
            tc.tile_pool(name="qt", bufs=1) as qtp,
            tc.tile_pool(name="outp", bufs=1) as outp,
            tc.tile_pool(name="psum", bufs=8, space="PSUM") as psum,
            tc.tile_pool(name="dram", bufs=1, space="DRAM") as dram,
        ):
            # ---- constants ------------------------------------------------
            # cblock cols: 0 ones, 1 eps, 2 prev, 3 allv, 4.. parts
            cblock = consts.tile([128, 4 + KA], F32)
            ones_col = cblock[:, 0:1]
            eps_t = cblock[:, 1:2]
            prev = cblock[:, 2:3]
            allv = cblock[:, 3:4]
            parts = cblock[:, 4:4 + KA]
            nc.vector.memset(ones_col, 1.0)
            nc.vector.memset(eps_t, EPS)
            ones_row = consts.tile([1, 128], F32)
            nc.vector.memset(ones_row, 1.0)
            # sblock cols: 0 s_mean, 1 s_clip, 2 s_bc, 3 t_bc, 4 nt_bc
            sblock = consts.tile([128, 5], F32)
            invb = consts.tile([128, N_STRIP], F32)  # inv[j]*s per strip

            s_bc = sblock[:, 2:3]

            # ---- phase W1: |w| partial over the held a-half ---------------
            # wa loads go FIRST on sync so their transfers win the DMA queues;
            # the partial reduce keeps pace on DVE; gpsimd stays empty so the
            # collective triggers the moment the partial lands.
            wa = wheld.tile([128, KA, DOUT_SH], F32)
            for i in range(KA):
                eng = nc.sync if i % 2 == 0 else nc.scalar
                eng.dma_start(out=wa[:, i, :],
                              in_=wa_d[i * 128:(i + 1) * 128, :])
            for i in range(KA):
                nc.vector.tensor_reduce(
                    parts[:, i:i + 1], wa[:, i, :],
                    axis=mybir.AxisListType.X,
                    op=mybir.AluOpType.add, apply_absolute_value=True)
            nc.vector.tensor_reduce(prev, parts, axis=mybir.AxisListType.X,
                                    op=mybir.AluOpType.add)


            # ---- strip prep ----------------------------------------------
            # xf DMA dispatch on sync AFTER cc_in so the wa transfers own the
            # DMA queues during W1. x16 does not depend on inv: the rmsnorm
            # chain (square/sqrt/recip) only feeds the drain scale invb[j].
            xT_tiles = {}
            deferred_inv = []

            def prep_strip(j, defer_inv=False):
                xb = f32s.tile([128, K], BF16, tag="xb", bufs=3, name=f"xb{j}")
                nc.sync.dma_start(out=xb, in_=x_d[j * 128:(j + 1) * 128, :])
                xsq = f32s.tile([128, K], mybir.dt.float8e5, tag="junk",
                                bufs=1, name=f"xsq{j}")
                sc = f32s.tile([128, 3], F32, tag="sc", bufs=8, name=f"sc{j}")
                ssq, rms, inv = sc[:, 0:1], sc[:, 1:2], sc[:, 2:3]
                nc.scalar.activation(xsq, xb,
                                     mybir.ActivationFunctionType.Square,
                                     accum_out=ssq)
                nc.scalar.activation(rms, ssq,
                                     mybir.ActivationFunctionType.Sqrt,
                                     bias=eps_t, scale=1.0 / K)
                nc.vector.reciprocal(inv, rms)
                if defer_inv:
                    # s_bc is not written yet at this emission point; the
                    # multiply must be emitted after the scale chain or it
                    # reads uninitialized SBUF on the first execution.
                    deferred_inv.append((inv, j))
                else:
                    nc.vector.tensor_tensor(invb[:, j:j + 1], inv, s_bc,
                                            mybir.AluOpType.mult)
                xT = f16s.tile([128, KT, 128], BF16, tag="xT", bufs=6,
                               name=f"xT{j}")
                nc.scalar.dma_start_transpose(out=xT, in_=xb)
                xT_tiles[j] = xT

            # collective: cc_in upload on sync (directly behind the wa
            # loads), trigger + readback on the otherwise-idle gpsimd queue
            cc_in = dram.tile([128, 1], F32)
            cc_out = dram.tile([128, 1], F32, addr_space="Shared")
            nc.sync.dma_start(out=cc_in, in_=prev)
            if use_cc:
                nc.gpsimd.collective_compute(
                    "AllReduce", mybir.AluOpType.add,
                    replica_groups=[list(range(N_CORES))],
                    ins=[cc_in.opt()], outs=[cc_out.opt()],
                )
            else:
                nc.gpsimd.dma_start(out=cc_out, in_=cc_in)

            for j in range(6):
                prep_strip(j, defer_inv=True)

            # ---- scale chain: s = max(sum/2^24, eps), thresholds +-s/2 ----
            # high_priority: the scheduler must slot these at the FRONT of
            # their engines' static order, not behind strip squares.
            prio = tc.high_priority()
            prio.__enter__()
            nc.gpsimd.dma_start(out=allv, in_=cc_out)
            if debug_fixed_scale is not None:
                nc.vector.memset(sblock[:, 2:3], debug_fixed_scale)
                nc.vector.memset(sblock[:, 3:4], debug_fixed_scale * 0.5)
                nc.vector.memset(sblock[:, 4:5], -debug_fixed_scale * 0.5)
            else:
                tot_ps = psum.tile([1, 1], F32, tag="mm")
                nc.tensor.matmul(tot_ps, lhsT=allv, rhs=ones_col,
                                 start=True, stop=True)
                nc.vector.tensor_scalar_mul(sblock[0:1, 0:1], tot_ps,
                                            1.0 / W_COUNT)
                nc.vector.tensor_scalar_max(sblock[0:1, 1:2],
                                            sblock[0:1, 0:1], EPS)
                s_bc_ps = psum.tile([128, 1], F32, tag="mm")
                nc.tensor.matmul(s_bc_ps, lhsT=ones_row, rhs=sblock[0:1, 1:2],
                                 start=True, stop=True)
                nc.vector.tensor_copy(out=sblock[:, 2:3], in_=s_bc_ps)
                nc.vector.tensor_scalar_mul(sblock[:, 3:4], sblock[:, 2:3],
                                            0.5)
                nc.vector.tensor_scalar_mul(sblock[:, 4:5], sblock[:, 2:3],
                                            -0.5)
            t_bc = sblock[:, 3:4]
            nt_bc = sblock[:, 4:5]
            prio.__exit__(None, None, None)
            for inv, j in deferred_inv:
                nc.vector.tensor_tensor(invb[:, j:j + 1], inv, s_bc,
                                        mybir.AluOpType.mult)
            deferred_inv.clear()

            # ---- quantize: q = (w > s/2) - (w < -s/2) ---------------------
            # pos + subtract on DVE, neg compare on GPSIMD, per-tile.
            # Tiles 0..KA-1 come from SBUF (wa); the rest stream via a
            # rotating pool whose first few DMAs prefetch during the
            # collective window.
            qQ = (qtp.tile([128, KT - nk8, DOUT_SH], BF16, name="qQ")
                  if nk8 < KT else None)
            qQ8 = (qtp.tile([128, nk8, DOUT_SH], F8, name="qQ8")
                   if nk8 > 0 else None)

            def q_dst(t):
                return qQ8[:, t, :] if t < nk8 else qQ[:, t - nk8, :]

            for t in range(KT):
                if t < KA:
                    src = wa[:, t, :]
                else:
                    wt = wstream.tile([128, DOUT_SH], F32, tag="wb", bufs=2,
                                      name=f"wb{t}")
                    nc.sync.dma_start(
                        out=wt, in_=wb_d[(t - KA) * 128:(t - KA + 1) * 128, :])
                    src = wt
                if t < nk8:
                    pos = f16s.tile([128, DOUT_SH], BF16, tag="pos", bufs=2,
                                    name=f"pos{t}")
                else:
                    pos = qQ[:, t - nk8, :]
                nc.vector.tensor_scalar(pos, src, t_bc, None,
                                        mybir.AluOpType.is_gt)
                neg = f16s.tile([128, DOUT_SH], BF16, tag="neg", bufs=2,
                                name=f"neg{t}")
                nc.vector.tensor_scalar(neg, src, nt_bc, -1.0,
                                        mybir.AluOpType.is_lt,
                                        mybir.AluOpType.mult)
                # pos += neg via accumulate-DMA (exact small ints); a
                # cast-DMA converts bf16 -> fp8 for the DoubleRow tiles
                nc.gpsimd.dma_start(out=pos, in_=neg,
                                    accum_op=mybir.AluOpType.add)
                if t < nk8:
                    nc.gpsimd.dma_start(out=q_dst(t), in_=pos)

            # ---- main loop: (strip, k-tile, dout-chunk) -------------------
            ND = DOUT_SH // 512  # 4 dout chunks of 512
            for j in range(N_STRIP):
                xT = xT_tiles[j]
                x8T = None
                if nk8 > 0:
                    x8T = f16s.tile([128, nk8 // 2, 2, 128], F8, tag="x8T",
                                    bufs=4, name=f"x8T{j}")
                    nc.vector.tensor_copy(out=x8T[:, :, :, :],
                                          in_=xT[:, 0:nk8, :])
                ps_d = [psum.tile([128, 512], F32, tag="mm",
                                  name=f"ps{j}_{d}") for d in range(ND)]
                # fp8 DoubleRow pair-matmuls for k-tiles [0, nk8):
                # lhsT [128,2,128] (2 k-tiles packed), rhs [128,2,512]
                # (free 1024 @ 2 elem/cycle), out [128,512] = one bank.
                for p2 in range(nk8 // 2):
                    for d in range(ND):
                        nc.tensor.matmul(
                            ps_d[d],
                            lhsT=x8T[:, p2, :, :],
                            rhs=qQ8[:, 2 * p2:2 * p2 + 2,
                                    d * 512:(d + 1) * 512],
                            start=(p2 == 0), stop=False,
                            perf_mode=mybir.MatmulPerfMode.DoubleRow,
                            skip_group_check=True)
                # fp16 for the rest
                for t in range(nk8, KT):
                    for d in range(ND):
                        nc.tensor.matmul(
                            ps_d[d], lhsT=xT[:, t, :],
                            rhs=qQ[:, t - nk8, d * 512:(d + 1) * 512],
                            start=(t == 0 and nk8 == 0), stop=(t == KT - 1),
                            skip_group_check=(nk8 > 0))
                for d in range(ND):
                    ob = outp.tile([128, 512], F32, tag="ob", bufs=4,
                                   name=f"ob{j}_{d}")
                    nc.scalar.activation(
                        out=ob, in_=ps_d[d],
                        func=mybir.ActivationFunctionType.Copy,
                        scale=invb[:, j:j + 1])
                    nc.scalar.dma_start(
                        out=out_d[j * 128:(j + 1) * 128,
                                  d * 512:(d + 1) * 512],
                        in_=ob)
                if j + 6 < N_STRIP:
                    prep_strip(j + 6)

    nc.compile()
    return nc


_NC_CACHE = {}


def _get_nc():
    if "nc" not in _NC_CACHE:
        _NC_CACHE["nc"] = build_nc()
    return _NC_CACHE["nc"]


def make_in_maps(x, weight, gamma):
    """Shard FULL inputs; k-halves swapped for rg=1 so the a-halves of
    w_shT partition the full weight exactly across the 8 cores. gamma is
    folded into x during sharding and the activations staged as bf16 (the
    on-device contraction runs in bf16 anyway)."""
    xf = np.asarray(x, np.float32).reshape(B * S, K)
    gamma = np.asarray(gamma, np.float32)
    xg = (xf * gamma).astype(ml_dtypes.bfloat16)
    wT = np.asarray(weight, np.float32).T  # [K, DOUT]
    half = K // 2
    in_maps = []
    for c in range(N_CORES):
        rg, cg = c // CG, c % CG
        wsh = wT[:, cg * DOUT_SH:(cg + 1) * DOUT_SH]
        xc = xg[rg * TOK_SH:(rg + 1) * TOK_SH]
        if rg == 0:
            wa, wb = wsh[:half], wsh[half:]
        else:
            wa, wb = wsh[half:], wsh[:half]
            xc = np.concatenate([xc[:, half:], xc[:, :half]], axis=1)
        in_maps.append({
            "x_sh": np.ascontiguousarray(xc),
            "w_a": np.ascontiguousarray(wa),
            "w_b": np.ascontiguousarray(wb),
        })
    return in_maps


def kernel(x, weight, gamma):
    in_maps = make_in_maps(x, weight, gamma)
    nc = _get_nc()
    res = run_bass_kernel_spmd(nc, in_maps, list(range(N_CORES))).results

    out = np.empty((B * S, DOUT), dtype=np.float32)
    for c in range(N_CORES):
        rg, cg = c // CG, c % CG
        out[rg * TOK_SH:(rg + 1) * TOK_SH,
            cg * DOUT_SH:(cg + 1) * DOUT_SH] = res[c]["out_sh"]
    return out.reshape(B, S, DOUT)


# revision 17
# speedup vs baseline: 1.0263x; 1.0154x over previous
"""BitLinear (RMSNorm + ternary-quantized linear) on 8 trn2 NeuronCores.

Reference math (fp32):
    xn   = x * rsqrt(mean(x^2, -1) + 1e-5) * gamma          # [B,S,K]
    s    = max(mean(|w|), 1e-5)                             # scalar
    q    = round(clip(w / s, -1, 1))                        # ternary {-1,0,1}
    out  = (xn @ q.T) * s                                   # [B,S,Dout]

Identities used by the kernel:
    q = (w > s/2) - (w < -s/2)   (exact, incl. round-half-even at |wn|=0.5)
    out[t,o] = inv[t] * s * sum_k (x[t,k]*gamma[k]) * q[o,k]
so gamma folds into x before the fp16 cast, q is exactly ternary in fp16,
and inv*s folds into the PSUM->SBUF epilogue. The contraction runs on the
PE in fp16 with fp32 PSUM accumulation.

Sharding: 2 token-groups x 4 dout-groups (core = rg*4 + cg).

Scale path (the startup critical path): each core's weight shard w_shT
[K, DOUT_SH] is split into an "a" half (k-tiles 0..7) and a "b" half
(k-tiles 8..15) such that the a-halves across the 8 cores partition the
FULL weight exactly once: rg=0 cores own original k [0,1024), rg=1 cores
own k [1024,2048). For rg=1 cores the host swaps the k-halves of x_sh,
gamma and w_shT so the program is identical on every core (the
contraction is invariant under a shared permutation of k). The a-half is
DMA'd first, |w| partial-reduced on DVE while it streams, AllReduced
(triggered from an otherwise-empty gpsimd queue at ~27us), and kept
resident in SBUF so quantization of k-tiles 0..7 needs no second read.

Main loop: per strip j (128 tokens), loop k-tile t outer / dout-chunk d
inner so one stationary xT load serves 4 N=512 matmuls (LDWEIGHTS
amortization). 8 PSUM banks = 2 strips in flight; drain via ACT copy
scaled by inv[j]*s; output DMA'd from gpsimd.

Optional fp8 split (NK8 > 0): k-tiles 0..NK8-1 run as e4m3 DoubleRow
pair-matmuls (2 k-tiles per stationary load, 2 MACs/cell/cycle), the
rest stay fp16. x and q for those tiles are exact-in-fp8 ternary /
rounded activations; the added output error was measured offline.
"""

import numpy as np
import ml_dtypes

import concourse.bass as bass
import concourse.tile as tile
from concourse import bacc, mybir
from concourse.bass_utils import run_bass_kernel_spmd

F32 = mybir.dt.float32
F16 = mybir.dt.float16
BF16 = mybir.dt.bfloat16
F8 = mybir.dt.float8e4

# Full-problem constants
B, S, K, DOUT = 4, 2048, 2048, 8192
N_CORES = 8
RG, CG = 2, 4  # token groups x dout groups
TOK_SH = (B * S) // RG     # 4096 tokens per core
DOUT_SH = DOUT // CG       # 2048 out-features per core
KT = K // 128              # 16 contraction tiles
KA = KT // 2               # tiles held + |w|-reduced per core (the "a" half)
N_STRIP = TOK_SH // 128    # 32 token strips
EPS = 1e-5
W_COUNT = float(DOUT * K)  # 16777216 = 2**24 (exact power of two)

NK8 = 8  # leading k-tiles computed in fp8 DoubleRow (0 = pure fp16)


def build_nc(nk8=NK8, use_cc=True, debug_fixed_scale=None):
    """Build the SPMD Bass program (one program, per-core inputs differ)."""
    nc = bacc.Bacc("TRN2", target_bir_lowering=False, num_devices=N_CORES)

    x_d = nc.declare_dram_parameter("x_sh", [TOK_SH, K], BF16, isOutput=False)
    wa_d = nc.declare_dram_parameter("w_a", [KA * 128, DOUT_SH], F32, isOutput=False)
    wb_d = nc.declare_dram_parameter("w_b", [(KT - KA) * 128, DOUT_SH], F32, isOutput=False)
    out_d = nc.declare_dram_parameter("out_sh", [TOK_SH, DOUT_SH], F32, isOutput=True)

    with tile.TileContext(nc, num_cores=N_CORES) as tc:
        with (
            tc.tile_pool(name="consts", bufs=1) as consts,
            tc.tile_pool(name="wheld", bufs=1) as wheld,
            tc.tile_pool(name="wstream", bufs=1) as wstream,
            tc.tile_pool(name="f32s", bufs=1) as f32s,
            tc.tile_pool(name="f16s", bufs=1) as f16s,
            tc.tile_pool(name="qt", bufs=1) as qtp,
            tc.tile_pool(name="outp", bufs=1) as outp,
            tc.tile_pool(name="psum", bufs=8, space="PSUM") as psum,
            tc.tile_pool(name="dram", bufs=1, space="DRAM") as dram,
        ):
            # ---- constants ------------------------------------------------
            # cblock cols: 0 ones, 1 eps, 2 prev, 3 allv, 4.. parts
            cblock = consts.tile([128, 4 + KA], F32)
            ones_col = cblock[:, 0:1]
            eps_t = cblock[:, 1:2]
            prev = cblock[:, 2:3]
            allv = cblock[:, 3:4]
            parts = cblock[:, 4:4 + KA]
            nc.vector.memset(ones_col, 1.0)
            nc.vector.memset(eps_t, EPS)
            ones_row = consts.tile([1, 128], F32)
            nc.vector.memset(ones_row, 1.0)
            # sblock cols: 0 s_mean, 1 s_clip, 2 s_bc, 3 t_bc, 4 nt_bc
            sblock = consts.tile([128, 5], F32)
            invb = consts.tile([128, N_STRIP], F32)  # inv[j]*s per strip

            s_bc = sblock[:, 2:3]

            # ---- phase W1: |w| partial over the held a-half ---------------
            # wa loads go FIRST on sync so their transfers win the DMA queues;
            # the partial reduce keeps pace on DVE; gpsimd stays empty so the
            # collective triggers the moment the partial lands.
            wa = wheld.tile([128, KA, DOUT_SH], F32)
            for i in range(KA):
                eng = nc.sync if i % 2 == 0 else nc.scalar
                eng.dma_start(out=wa[:, i, :],
                              in_=wa_d[i * 128:(i + 1) * 128, :])
            for i in range(KA):
                nc.vector.tensor_reduce(
                    parts[:, i:i + 1], wa[:, i, :],
                    axis=mybir.AxisListType.X,
                    op=mybir.AluOpType.add, apply_absolute_value=True)
            nc.vector.tensor_reduce(prev, parts, axis=mybir.AxisListType.X,
                                    op=mybir.AluOpType.add)


            # ---- strip prep ----------------------------------------------
            # xf DMA dispatch on sync AFTER cc_in so the wa transfers own the
            # DMA queues during W1. x16 does not depend on inv: the rmsnorm
            # chain (square/sqrt/recip) only feeds the drain scale invb[j].
            xT_tiles = {}
            deferred_inv = []

            def prep_strip(j, defer_inv=False):
                xb = f32s.tile([128, K], BF16, tag="xb", bufs=3, name=f"xb{j}")
                nc.sync.dma_start(out=xb, in_=x_d[j * 128:(j + 1) * 128, :])
                xsq = f32s.tile([128, K], mybir.dt.float8e5, tag="junk",
                                bufs=1, name=f"xsq{j}")
                sc = f32s.tile([128, 3], F32, tag="sc", bufs=8, name=f"sc{j}")
                ssq, rms, inv = sc[:, 0:1], sc[:, 1:2], sc[:, 2:3]
                nc.scalar.activation(xsq, xb,
                                     mybir.ActivationFunctionType.Square,
                                     accum_out=ssq)
                nc.scalar.activation(rms, ssq,
                                     mybir.ActivationFunctionType.Sqrt,
                                     bias=eps_t, scale=1.0 / K)
                nc.vector.reciprocal(inv, rms)
                if defer_inv:
                    # s_bc is not written yet at this emission point; the
                    # multiply must be emitted after the scale chain or it
                    # reads uninitialized SBUF on the first execution.
                    deferred_inv.append((inv, j))
                else:
                    nc.vector.tensor_tensor(invb[:, j:j + 1], inv, s_bc,
                                            mybir.AluOpType.mult)
                xT = f16s.tile([128, KT, 128], BF16, tag="xT", bufs=6,
                               name=f"xT{j}")
                nc.scalar.dma_start_transpose(out=xT, in_=xb)
                xT_tiles[j] = xT

            # Partition-reduce + broadcast BEFORE the collective (PE is idle
            # here), so after the AllReduce only a readback and three tiny
            # DVE ops remain on the critical path.
            tot_ps = psum.tile([1, 1], F32, tag="mm")
            nc.tensor.matmul(tot_ps, lhsT=prev, rhs=ones_col,
                             start=True, stop=True)
            tot_sb = consts.tile([1, 1], F32, name="tot_sb")
            nc.vector.tensor_copy(out=tot_sb, in_=tot_ps)
            rep_ps = psum.tile([128, 1], F32, tag="mm")
            nc.tensor.matmul(rep_ps, lhsT=ones_row, rhs=tot_sb,
                             start=True, stop=True)
            repl = consts.tile([128, 1], F32, name="repl")
            nc.vector.tensor_copy(out=repl, in_=rep_ps)

            # collective: cc_in upload on sync (directly behind the wa
            # loads), trigger + readback on the otherwise-idle gpsimd queue
            cc_in = dram.tile([128, 1], F32)
            cc_out = dram.tile([128, 1], F32, addr_space="Shared")
            nc.sync.dma_start(out=cc_in, in_=repl)
            if use_cc:
                nc.gpsimd.collective_compute(
                    "AllReduce", mybir.AluOpType.add,
                    replica_groups=[list(range(N_CORES))],
                    ins=[cc_in.opt()], outs=[cc_out.opt()],
                )
            else:
                nc.gpsimd.dma_start(out=cc_out, in_=cc_in)

            for j in range(6):
                prep_strip(j, defer_inv=True)

            # ---- scale chain: s = max(sum/2^24, eps), thresholds +-s/2 ----
            # high_priority: the scheduler must slot these at the FRONT of
            # their engines' static order, not behind strip squares.
            prio = tc.high_priority()
            prio.__enter__()
            nc.gpsimd.dma_start(out=allv, in_=cc_out)
            if debug_fixed_scale is not None:
                nc.vector.memset(sblock[:, 2:3], debug_fixed_scale)
                nc.vector.memset(sblock[:, 3:4], debug_fixed_scale * 0.5)
                nc.vector.memset(sblock[:, 4:5], -debug_fixed_scale * 0.5)
            else:
                # allv = sum of replicated locals = total, per partition.
                # s = max(total/2^24, eps); thresholds +-s/2. (2^-24 scaling
                # and halving are exact exponent shifts.)
                nc.vector.tensor_scalar(sblock[:, 2:3], allv, 1.0 / W_COUNT,
                                        EPS, mybir.AluOpType.mult,
                                        mybir.AluOpType.max)
                nc.vector.tensor_scalar_mul(sblock[:, 3:4], sblock[:, 2:3],
                                            0.5)
                nc.vector.tensor_scalar_mul(sblock[:, 4:5], sblock[:, 2:3],
                                            -0.5)
            t_bc = sblock[:, 3:4]
            nt_bc = sblock[:, 4:5]
            prio.__exit__(None, None, None)
            for inv, j in deferred_inv:
                nc.vector.tensor_tensor(invb[:, j:j + 1], inv, s_bc,
                                        mybir.AluOpType.mult)
            deferred_inv.clear()

            # ---- quantize: q = (w > s/2) - (w < -s/2) ---------------------
            # pos + subtract on DVE, neg compare on GPSIMD, per-tile.
            # Tiles 0..KA-1 come from SBUF (wa); the rest stream via a
            # rotating pool whose first few DMAs prefetch during the
            # collective window.
            qQ = (qtp.tile([128, KT - nk8, DOUT_SH], BF16, name="qQ")
                  if nk8 < KT else None)
            qQ8 = (qtp.tile([128, nk8, DOUT_SH], F8, name="qQ8")
                   if nk8 > 0 else None)

            def q_dst(t):
                return qQ8[:, t, :] if t < nk8 else qQ[:, t - nk8, :]

            for t in range(KT):
                if t < KA:
                    src = wa[:, t, :]
                else:
                    wt = wstream.tile([128, DOUT_SH], F32, tag="wb", bufs=2,
                                      name=f"wb{t}")
                    nc.sync.dma_start(
                        out=wt, in_=wb_d[(t - KA) * 128:(t - KA + 1) * 128, :])
                    src = wt
                if t < nk8:
                    pos = f16s.tile([128, DOUT_SH], BF16, tag="pos", bufs=3,
                                    name=f"pos{t}")
                else:
                    pos = qQ[:, t - nk8, :]
                nc.vector.tensor_scalar(pos, src, t_bc, None,
                                        mybir.AluOpType.is_gt)
                neg = f16s.tile([128, DOUT_SH], BF16, tag="neg", bufs=3,
                                name=f"neg{t}")
                nc.vector.tensor_scalar(neg, src, nt_bc, -1.0,
                                        mybir.AluOpType.is_lt,
                                        mybir.AluOpType.mult)
                # pos += neg via accumulate-DMA (exact small ints); a
                # cast-DMA converts bf16 -> fp8 for the DoubleRow tiles
                nc.gpsimd.dma_start(out=pos, in_=neg,
                                    accum_op=mybir.AluOpType.add)
                if t < nk8:
                    nc.gpsimd.dma_start(out=q_dst(t), in_=pos)

            # ---- main loop: (strip, k-tile, dout-chunk) -------------------
            ND = DOUT_SH // 512  # 4 dout chunks of 512
            for j in range(N_STRIP):
                xT = xT_tiles[j]
                x8T = None
                if nk8 > 0:
                    x8T = f16s.tile([128, nk8 // 2, 2, 128], F8, tag="x8T",
                                    bufs=4, name=f"x8T{j}")
                    nc.vector.tensor_copy(out=x8T[:, :, :, :],
                                          in_=xT[:, 0:nk8, :])
                ps_d = [psum.tile([128, 512], F32, tag="mm",
                                  name=f"ps{j}_{d}") for d in range(ND)]
                # fp8 DoubleRow pair-matmuls for k-tiles [0, nk8):
                # lhsT [128,2,128] (2 k-tiles packed), rhs [128,2,512]
                # (free 1024 @ 2 elem/cycle), out [128,512] = one bank.
                for p2 in range(nk8 // 2):
                    for d in range(ND):
                        nc.tensor.matmul(
                            ps_d[d],
                            lhsT=x8T[:, p2, :, :],
                            rhs=qQ8[:, 2 * p2:2 * p2 + 2,
                                    d * 512:(d + 1) * 512],
                            start=(p2 == 0), stop=False,
                            perf_mode=mybir.MatmulPerfMode.DoubleRow,
                            skip_group_check=True)
                # fp16 for the rest
                for t in range(nk8, KT):
                    for d in range(ND):
                        nc.tensor.matmul(
                            ps_d[d], lhsT=xT[:, t, :],
                            rhs=qQ[:, t - nk8, d * 512:(d + 1) * 512],
                            start=(t == 0 and nk8 == 0), stop=(t == KT - 1),
                            skip_group_check=(nk8 > 0))
                for d in range(ND):
                    ob = outp.tile([128, 512], F32, tag="ob", bufs=4,
                                   name=f"ob{j}_{d}")
                    nc.scalar.activation(
                        out=ob, in_=ps_d[d],
                        func=mybir.ActivationFunctionType.Copy,
                        scale=invb[:, j:j + 1])
                    nc.scalar.dma_start(
                        out=out_d[j * 128:(j + 1) * 128,
                                  d * 512:(d + 1) * 512],
                        in_=ob)
                if j + 6 < N_STRIP:
                    prep_strip(j + 6)

    nc.compile()
    return nc


_NC_CACHE = {}


def _get_nc():
    if "nc" not in _NC_CACHE:
        _NC_CACHE["nc"] = build_nc()
    return _NC_CACHE["nc"]


def make_in_maps(x, weight, gamma):
    """Shard FULL inputs; k-halves swapped for rg=1 so the a-halves of
    w_shT partition the full weight exactly across the 8 cores. gamma is
    folded into x during sharding and the activations staged as bf16 (the
    on-device contraction runs in bf16 anyway)."""
    xf = np.asarray(x, np.float32).reshape(B * S, K)
    gamma = np.asarray(gamma, np.float32)
    xg = (xf * gamma).astype(ml_dtypes.bfloat16)
    wT = np.asarray(weight, np.float32).T  # [K, DOUT]
    half = K // 2
    in_maps = []
    for c in range(N_CORES):
        rg, cg = c // CG, c % CG
        wsh = wT[:, cg * DOUT_SH:(cg + 1) * DOUT_SH]
        xc = xg[rg * TOK_SH:(rg + 1) * TOK_SH]
        if rg == 0:
            wa, wb = wsh[:half], wsh[half:]
        else:
            wa, wb = wsh[half:], wsh[:half]
            xc = np.concatenate([xc[:, half:], xc[:, :half]], axis=1)
        in_maps.append({
            "x_sh": np.ascontiguousarray(xc),
            "w_a": np.ascontiguousarray(wa),
            "w_b": np.ascontiguousarray(wb),
        })
    return in_maps


def kernel(x, weight, gamma):
    in_maps = make_in_maps(x, weight, gamma)
    nc = _get_nc()
    res = run_bass_kernel_spmd(nc, in_maps, list(range(N_CORES))).results

    out = np.empty((B * S, DOUT), dtype=np.float32)
    for c in range(N_CORES):
        rg, cg = c // CG, c % CG
        out[rg * TOK_SH:(rg + 1) * TOK_SH,
            cg * DOUT_SH:(cg + 1) * DOUT_SH] = res[c]["out_sh"]
    return out.reshape(B, S, DOUT)
